# revision 44
# baseline (speedup 1.0000x reference)
"""Trainium2 Bass kernel for a single attention head with query-axis softmax.

Reference semantics (per batch b):
    k = x @ Wk; q = x @ Wq; v = x @ Wv                 # [T, H]
    wei = (q @ k^T) * E**-0.5                          # [T(query), T(key)]
    wei = where(tril, wei, -inf)                       # causal: keep s <= t
    p = softmax(wei, axis=0 over query t)              # NOTE: query axis!
    out = p @ v                                        # [T, H]

Because the softmax normalizes over the query axis t (per key column s),
out[t,h] = sum_s E[t,s] * v[s,h] / d[s] with E[t,s] = exp(wei[t,s])
(zero for s > t) and d[s] = sum_t E[t,s].  The kernel computes E^T tiles
([s on partitions, t free]) so d is a free-axis row sum (fused into the
exp instruction via accum_out), scales v rows by 1/d, and accumulates
out^T on PE.  Cross-partition layout fixes (v^T -> v, out^T -> out) are
PE transposes against a DMA-loaded identity.

PE-array packing: the S matmuls contract over only H=64 partitions, so
two consecutive t-blocks run CONCURRENTLY in the two row-halves of the
128x128 array (row_grp h0/h64).  k^T is duplicated into both partition
halves for free by packing the k projection weights as [Wk|Wk]; q lands
in the partition half matching its block parity by alternating the
packed projection weights [Wq|Wv] (even steps) / [Wv|Wq] (odd steps).
The AV matmuls already pair the same way in the column halves (out^T
rows 0:64 / 64:128 by block parity).

Sharding: batch dim (8) across the 8 NeuronCores, weights replicated.
The host passes x pre-transposed per batch (xT[b] = x[b].T) in bf16 so
no on-device transpose of the big activation tensor is needed; matmul
operands are bf16 with fp32 PSUM accumulation.
"""

import numpy as np
import ml_dtypes

import concourse.bass as bass
import concourse.tile as tile
from concourse import bacc, mybir
from concourse import bass_utils

B, T, E, H = 8, 2048, 1024, 64
P = 128                       # partitions
CB = 512                      # column block (t) width
NE = E // P                   # 8 contraction chunks for projections
NJ = T // CB                  # 4 column blocks
NI = T // P                   # 16 s-chunks
SCALE = float(E) ** -0.5      # note: embed**-0.5, not head_size**-0.5
MASK_NEG = -1.0e30
F32 = mybir.dt.float32
F32R = mybir.dt.float32r
BF16 = mybir.dt.bfloat16
X = mybir.AxisListType.X
EXP = mybir.ActivationFunctionType.Exp


def _qhalf(j):
    """Partition half (0 or 64) where q block j lives (by step parity)."""
    return H if (j % 2) else 0


def _vhalf(j):
    return 0 if (j % 2) else H


def _emit(tc, xT_d, wA_even_d, wA_odd_d, wkk_d, masks_d, identb_d, out_d):
    nc = tc.nc
    from contextlib import ExitStack

    with ExitStack() as ctx:
        singles = ctx.enter_context(tc.tile_pool(name="singles", bufs=1))
        xpool = ctx.enter_context(tc.tile_pool(name="x", bufs=1))
        epool = ctx.enter_context(tc.tile_pool(name="erow", bufs=9))
        dpool = ctx.enter_context(tc.tile_pool(name="dsmall", bufs=12))
        vpool = ctx.enter_context(tc.tile_pool(name="vrow", bufs=9))
        # one unified 3-slot rotation for ALL 2-bank psum tiles (S rows,
        # proj chains, transposes, finale): 3x2 banks + pout 2 = 8 banks.
        # A proj chain holds one slot for its step; the other two give the
        # S pipeline a true row of lookahead in every phase.
        ps = ctx.enter_context(tc.tile_pool(name="ps", bufs=3, space="PSUM"))
        pout = ctx.enter_context(tc.tile_pool(name="pout", bufs=1, space="PSUM"))

        # x^T viewed as [partition, e-chunk, t].  The j=3 column is split
        # into two DMAs (own tiles) so the first projection chain can
        # start as soon as e-chunks 0..3 land; remaining columns follow
        # in consumption order on the same HWDGE ring (FIFO drain).
        # junk memset first: nothing may delay the PE warmup matmuls
        junk = singles.tile([P, CB], BF16)
        nc.gpsimd.memset(junk[:], 1.0)

        # ALL inputs go on ONE HWDGE ring (sync) in exact priority order:
        # HBM bandwidth is the limit, so queue-splitting only scrambles
        # arrival order (queues round-robin at packet granularity).  The
        # ring drains FIFO, so each tensor lands as early as possible
        # given everything more urgent before it.
        xT_v = xT_d.rearrange("(ne p) t -> p ne t", p=P)
        wA_odd = singles.tile([P, NE * 2 * H], BF16)
        nc.sync.dma_start(out=wA_odd[:], in_=wA_odd_d[:])
        xts3a = xpool.tile([P, 4, CB], BF16, tag="xt3a", name="xts3a")
        nc.sync.dma_start(out=xts3a[:], in_=xT_v[:, 0:4, 3 * CB :])
        wkk = singles.tile([P, NE * 2 * H], BF16)
        nc.sync.dma_start(out=wkk[:], in_=wkk_d[:])
        xts3b = xpool.tile([P, NE - 4, CB], BF16, tag="xt3b", name="xts3b")
        nc.sync.dma_start(out=xts3b[:], in_=xT_v[:, 4:, 3 * CB :])
        wA_even = singles.tile([P, NE * 2 * H], BF16)
        nc.sync.dma_start(out=wA_even[:], in_=wA_even_d[:])
        masks_sb = singles.tile([P, 4 * P], F32)
        nc.sync.dma_start(out=masks_sb[:], in_=masks_d[:])
        identb = singles.tile([P, P], BF16)
        nc.sync.dma_start(out=identb[:], in_=identb_d[:])
        xts2 = xpool.tile([P, NE, CB], BF16, tag="xt2", name="xts2")
        nc.sync.dma_start(out=xts2[:], in_=xT_v[:, :, 2 * CB : 3 * CB])
        xtsr = xpool.tile([P, NE, 2 * CB], BF16, tag="xtr", name="xtsr")
        nc.sync.dma_start(out=xtsr[:], in_=xT_v[:, :, 0 : 2 * CB])

        def x_rhs(e, j):
            if j == 3:
                return xts3a[:, e, :] if e < 4 else xts3b[:, e - 4, :]
            if j == 2:
                return xts2[:, e, :]
            return xtsr[:, e, j * CB : (j + 1) * CB]

        # dummy matmuls while the first DMAs land: primes the PE
        # activity monitor so the real chains start at 2.4 GHz, not 1.2
        pwarm = ps.tile([P, 2 * CB], F32, tag="ps", name="pwarm")
        NWARM = 9
        for w in range(NWARM):
            nc.tensor.matmul(
                pwarm[:, 0:CB],
                lhsT=junk[:, 0:P],
                rhs=junk[:],
                start=(w == 0),
                stop=(w == NWARM - 1),
            )
        # tiny dummy exp: walrus hoists the ~1.3us ACT table load here
        # (ACT is idle) instead of paying it before the first real exp
        dwarm = dpool.tile([P, 1], F32, tag="dwarm")
        nc.scalar.activation(out=dwarm[:], in_=junk[:, 0:1], func=EXP)

        # persistent activations: q blocks / k^T copies live in the
        # partition half given by block parity (k^T in BOTH halves)
        q_sb = singles.tile([P, T], BF16)
        kT_sb = singles.tile([P, T], BF16)
        vT_sb = singles.tile([P, T], BF16)   # v^T chunk j in half _vhalf(j)
        outT_sb = singles.tile([P, T // 2], BF16)  # rows 0:64 jj even, 64:128 odd

        # out^T accumulators packed 2 per bank: jj even rows 0:64, odd 64:128.
        # Accumulation groups on disjoint partition ranges of one bank are
        # fine on HW (per-element has_written); skip the sim's coarse check.
        pout_tiles = [
            pout.tile([P, CB], F32, tag=f"pt{a}", name=f"pt{a}") for a in range(2)
        ]

        def pout_slice(jj, c0, c1):
            rb = H * (jj % 2)
            return pout_tiles[jj // 2][rb : rb + H, c0:c1]

        # deferred AV emission (lag one row behind S so PE never waits on
        # the d / v' chain): each entry = (r, d0, erow, vi, j_of_row)
        pending_av = []

        def _av_one(rj, d0, erow, vi, jj):
            c = (jj - rj[1]) * CB
            lo = d0 if jj == rj[1] else 0
            nc.tensor.matmul(
                pout_slice(jj, lo, CB),
                lhsT=vi[:],
                rhs=erow[:, c + lo : c + CB],
                start=(jj == rj[1] and rj[0] == 0),
                stop=(rj[1] == 0 and rj[0] == 3),
                skip_group_check=True,
            )

        def flush_av(final):
            if final:
                # group by output bank so bank A closes early and its
                # finale transposes overlap bank B's last matmuls
                rows = list(pending_av)
                pending_av.clear()
                for jj in range(NJ):
                    for rj, d0, erow, vi in rows:
                        if jj >= rj[1]:
                            _av_one(rj, d0, erow, vi, jj)
                    if jj % 2 == 1:
                        finale_bank(jj // 2)
                return
            rj, d0, erow, vi = pending_av.pop(0)
            for jj in range(rj[1], NJ):
                _av_one(rj, d0, erow, vi, jj)

        # projection matmul emission is spread through the PREVIOUS step's
        # rows so the PE instruction stream stays dense (HAM stays warm).
        # Chain A ([Wq|Wv] or [Wv|Wq] by parity) and chain B ([Wk|Wk])
        # interleave so the e-chunk DMA chase at j=3 has a longer bridge.
        def proj_thunks(j):
            pproj = ps.tile([P, 2 * CB], F32, tag="ps", name="pproj")
            wA = wA_odd if (j % 2) else wA_even
            thunks = []
            for e in range(NE):
                thunks.append(
                    lambda e=e: nc.tensor.matmul(
                        pproj[:, 0:CB],
                        lhsT=wA[:, e * 2 * H : (e + 1) * 2 * H],
                        rhs=x_rhs(e, j),
                        start=(e == 0),
                        stop=(e == NE - 1),
                    )
                )
                thunks.append(
                    lambda e=e: nc.tensor.matmul(
                        pproj[:, CB : 2 * CB],
                        lhsT=wkk[:, e * 2 * H : (e + 1) * 2 * H],
                        rhs=x_rhs(e, j),
                        start=(e == 0),
                        stop=(e == NE - 1),
                    )
                )
            return pproj, thunks

        def proj_casts(j, pproj, use_act=False):
            # vT first (feeds the next step's up-front transposes); the
            # initial (j=3) call splits across ACT+DVE since ACT is idle
            t0 = j * CB
            qh, vh = _qhalf(j), _vhalf(j)
            nc.vector.tensor_copy(
                vT_sb[vh : vh + H, t0 : t0 + CB], pproj[vh : vh + H, 0:CB]
            )
            nc.scalar.copy(kT_sb[:, t0 : t0 + CB], pproj[:, CB : 2 * CB])
            nc.vector.tensor_copy(
                q_sb[qh : qh + H, t0 : t0 + CB], pproj[qh : qh + H, 0:CB]
            )

        # --- main pipeline: column blocks in descending order --------------
        # chains of pending projection matmuls, dripped through the row
        # loop; when one drains its casts are emitted immediately and the
        # next chain (if its x data can be in flight) is queued so late
        # steps carry less projection work
        next_proj = []
        drip_state = {"pproj": None, "j": None, "cast_done": True}

        def start_chain(j):
            pproj_c, thunks = proj_thunks(j)
            next_proj.extend(thunks)
            drip_state.update(pproj=pproj_c, j=j, cast_done=False)

        def drip_proj(k):
            for _ in range(min(k, len(next_proj))):
                next_proj.pop(0)()
            if not next_proj and not drip_state["cast_done"]:
                # chain fully emitted: emit its casts right away so the
                # next step's S matmuls find q/kT ready at the boundary
                drip_state["cast_done"] = True
                proj_casts(drip_state["j"], drip_state["pproj"])
                if drip_state["j"] == 1:
                    # queue the j=0 chain now: its x data (xtsr) is long
                    # landed and j=1's rows are ACT-bound without it
                    start_chain(0)

        pproj, thunks = proj_thunks(3)
        for t in thunks:
            t()
        proj_casts(3, pproj, use_act=True)

        for j in reversed(range(NJ)):
            if j > 0 and drip_state["cast_done"] and drip_state["j"] != j - 1:
                start_chain(j - 1)

            vi_raws = []

            # rows i = 4j .. 4j+3 of E^T are now computable in full
            for r in range(4):
                i = 4 * j + r
                s0 = i * P
                d0 = r * P  # first unmasked column of the diagonal block
                nblk = NJ - j
                # flush due AV matmuls BEFORE this row's S pair: their
                # operands are long ready, so they fill the PE bubble
                # while S waits for its psum slot instead of queuing
                # behind it (engines run strictly in order).  At a step
                # boundary (r=0) flush only the OLDEST pending row: it
                # fills the boundary bubble, while the previous row's vi
                # is still in flight and would stall the queue.
                if r == 0:
                    if len(pending_av) >= 2:
                        flush_av(False)
                else:
                    while len(pending_av) >= 2:
                        flush_av(False)
                erow = epool.tile([P, T], BF16)
                dparts = dpool.tile([P, 2], F32, tag="dparts")
                # pair layout: consecutive blocks share a 2-bank tile
                # (opposite parity -> concurrent S).  At j=1 the diagonal
                # block goes ALONE first: its (short, masked) activation
                # releases the psum slot early and shortens the row spine.
                if j == 1:
                    pairs = [(1, CB), (2, 2 * CB)]
                else:
                    pairs = [
                        (j + 2 * p, CB * min(2, NJ - (j + 2 * p)))
                        for p in range((nblk + 1) // 2)
                    ]
                npair = len(pairs)
                use_accum = True
                for pair, (jj0, w) in enumerate(pairs):
                    pst = ps.tile([P, 2 * CB], F32, tag="ps")
                    for u in range(w // CB):
                        jj = jj0 + u
                        lo = d0 if jj == j else 0
                        h = H * (jj % 2)
                        nc.tensor.matmul(
                            pst[:, u * CB + lo : (u + 1) * CB],
                            lhsT=kT_sb[h : h + H, s0 : s0 + P],
                            rhs=q_sb[h : h + H, jj * CB + lo : (jj + 1) * CB],
                            start=True,
                            stop=True,
                        )
                    # step 3 row 12: don't drip yet — the j=2 chain waits
                    # on the xts2 DMA and would head-of-line block the
                    # row matmuls behind it in the PE queue
                    if not (j == 3 and r < 1):
                        drip_proj(3)
                    lo = d0 if pair == 0 else 0
                    if pair == 0:
                        # additive -1e30 triangle on the partial subblock
                        nc.vector.tensor_add(
                            pst[:, lo : lo + P],
                            pst[:, lo : lo + P],
                            masks_sb[:, r * P : (r + 1) * P],
                        )
                    c = (jj0 - j) * CB
                    nc.scalar.activation(
                        out=erow[:, c + lo : c + w],
                        in_=pst[:, lo:w],
                        func=EXP,
                        scale=SCALE,
                        accum_out=(
                            dparts[:, pair : pair + 1] if use_accum else None
                        ),
                    )

                # After row 0's S/exp are queued, stage this step's four
                # v transposes: the boundary S-pair enters the PE queue
                # with nothing in front of it, the transposes fill PE
                # behind it, and each copy lands before its row's d-chain
                # needs it.  ACT does the psum->sbuf evacuation (idle gap
                # behind row 0's exp); slot recycling resolves against
                # row 0's own exps.
                if r == 0:
                    vh = _vhalf(j)
                    for rt in range(4):
                        s0t = (4 * j + rt) * P
                        pvt = ps.tile([P, 2 * CB], BF16, tag="ps", name="pvt")
                        nc.tensor.transpose(
                            pvt[:, 0:H],
                            vT_sb[vh : vh + H, s0t : s0t + P],
                            identb[vh : vh + H, vh : vh + H],
                        )
                        vr = vpool.tile([P, H], BF16, tag="vr", name="vi_raw")
                        nc.scalar.copy(vr[:], pvt[:, 0:H])
                        vi_raws.append(vr)

                # d = sum over the row; 1/d feeds the v' scale
                dinv = dpool.tile([P, 1], F32, tag="dinv")
                if not use_accum:
                    dsum = dpool.tile([P, 1], F32, tag="dsum")
                    nc.vector.reduce_sum(
                        dsum[:], erow[:, d0 : CB * nblk], axis=X
                    )
                    nc.vector.reciprocal(dinv[:], dsum[:])
                elif npair > 1:
                    dsum = dpool.tile([P, 1], F32, tag="dsum")
                    nc.vector.reduce_sum(dsum[:], dparts[:, 0:npair], axis=X)
                    nc.vector.reciprocal(dinv[:], dsum[:])
                else:
                    nc.vector.reciprocal(dinv[:], dparts[:, 0:1])

                vi = vpool.tile([P, H], BF16, tag="vi", name="vi")
                nc.vector.tensor_scalar_mul(vi[:], vi_raws[r][:], dinv[:])

                if not (j == 3 and r < 1):
                    drip_proj(3)
                pending_av.append(((r, j), d0, erow, vi))

            # drain remaining next-step projection matmuls (+ casts)
            drip_proj(len(next_proj))

        # finale per pout bank: cast out of PSUM, transpose to natural
        # layout, stage for the single store DMA
        onf = singles.tile([P, NI, H], F32)

        out_v = out_d.rearrange("(c p) h -> p c h", p=P)

        def finale_bank(a):
            nc.vector.tensor_copy(
                outT_sb[:, a * CB : (a + 1) * CB], pout_tiles[a][:]
            )
            # two psum tiles, 4 transposes each at distinct column
            # offsets, one wide cast per tile — avoids the serial
            # transpose/cast rotation of the old per-chunk path
            for half in range(2):
                pso = ps.tile([P, 2 * CB], BF16, tag="ps", name="pso")
                for cc in range(4):
                    c = 8 * a + 4 * half + cc
                    jj = c // 4
                    rb = H * (jj % 2)
                    col = (jj // 2) * CB + (c % 4) * P
                    nc.tensor.transpose(
                        pso[:, cc * H : (cc + 1) * H],
                        outT_sb[rb : rb + H, col : col + P],
                        identb[rb : rb + H, rb : rb + H],
                    )
                nc.vector.tensor_copy(
                    onf[:, 8 * a + 4 * half : 8 * a + 4 * half + 4, :],
                    pso[:, 0 : 4 * H],
                )
                # store each quarter as soon as it's staged so the last
                # DMA (and its ~2us completion receipt) is small and early
                nc.sync.dma_start(
                    out=out_v[:, 8 * a + 4 * half : 8 * a + 4 * half + 4, :],
                    in_=onf[:, 8 * a + 4 * half : 8 * a + 4 * half + 4, :],
                )

        flush_av(True)


def _build_program():
    nc = bacc.Bacc("TRN2", target_bir_lowering=False, debug=False, num_devices=B)
    xT_d = nc.dram_tensor("xT", [E, T], BF16, kind="ExternalInput").ap()
    wA_even_d = nc.dram_tensor(
        "wA_even", [P, NE * 2 * H], BF16, kind="ExternalInput"
    ).ap()
    wA_odd_d = nc.dram_tensor(
        "wA_odd", [P, NE * 2 * H], BF16, kind="ExternalInput"
    ).ap()
    wkk_d = nc.dram_tensor("wkk", [P, NE * 2 * H], BF16, kind="ExternalInput").ap()
    masks_d = nc.dram_tensor("masks", [P, 4 * P], F32, kind="ExternalInput").ap()
    identb_d = nc.dram_tensor("identb", [P, P], BF16, kind="ExternalInput").ap()
    out_d = nc.dram_tensor("out", [T, H], F32, kind="ExternalOutput").ap()
    with tile.TileContext(nc) as tc:
        _emit(tc, xT_d, wA_even_d, wA_odd_d, wkk_d, masks_d, identb_d, out_d)
    nc.compile()
    return nc


def _host_masks():
    """[128, 4*128]: additive triangle mask r at cols [128r, 128(r+1))."""
    m = np.full((P, 4 * P), MASK_NEG, dtype=np.float32)
    p = np.arange(P)[:, None]
    f = np.arange(P)[None, :]
    for r in range(4):
        m[:, r * P : (r + 1) * P][f >= p] = 0.0
    return m


def _host_inputs(x, Wk, Wq, Wv):
    bf = ml_dtypes.bfloat16
    x = np.asarray(x, dtype=np.float32)
    xT = np.ascontiguousarray(np.transpose(x, (0, 2, 1))).astype(bf)  # [B, E, T]

    def pack_w(*ws):
        # [E, h_tot] (concat) -> [128, NE * h_tot]: chunk e at cols e*h_tot
        w = np.concatenate([np.asarray(a, np.float32) for a in ws], axis=1)
        h = w.shape[1]
        return np.ascontiguousarray(
            w.reshape(NE, P, h).transpose(1, 0, 2).reshape(P, NE * h)
        ).astype(bf)

    wA_even = pack_w(Wq, Wv)   # q in partitions 0:64 (even blocks)
    wA_odd = pack_w(Wv, Wq)    # q in partitions 64:128 (odd blocks)
    wkk = pack_w(Wk, Wk)       # k^T duplicated into both halves
    masks = _host_masks()
    identb = np.eye(P, dtype=np.float32).astype(bf)
    return [
        {
            "xT": xT[b],
            "wA_even": wA_even,
            "wA_odd": wA_odd,
            "wkk": wkk,
            "masks": masks,
            "identb": identb,
        }
        for b in range(B)
    ]


def _ensure_axon_ntff_hook():
    """The agent image's antenv lacks axon_hooks; synthesize it so
    run_bass_kernel_spmd's trace path can find the NTFF profile hook."""
    import sys
    import types

    if "antenv.axon_hooks" in sys.modules:
        return
    try:
        import antenv

        mod = types.ModuleType("antenv.axon_hooks")
        mod._hook = None

        def set_axon_ntff_profile_hook(h):
            mod._hook = h

        def get_axon_ntff_profile_hook():
            return mod._hook

        mod.set_axon_ntff_profile_hook = set_axon_ntff_profile_hook
        mod.get_axon_ntff_profile_hook = get_axon_ntff_profile_hook
        sys.modules["antenv.axon_hooks"] = mod
        antenv.axon_hooks = mod

        from trn_agent_boot.trn_boot import _ntff_profile_via_ctypes

        hook = _ntff_profile_via_ctypes("/opt/axon/libaxon_pjrt.so")
        if hook is not None:
            mod._hook = hook
    except Exception as e:  # degrade to untraced run
        print(f"NTFF hook setup failed ({e}); tracing will be skipped")


def kernel(x, Wk, Wq, Wv, _trace=False, _trace_kwargs=None):
    if _trace:
        _ensure_axon_ntff_hook()
    in_maps = _host_inputs(x, Wk, Wq, Wv)
    nc = _build_program()
    res = bass_utils.run_bass_kernel_spmd(
        nc, in_maps, list(range(B)), trace=_trace, **(_trace_kwargs or {})
    )
    out = np.stack([res.results[b]["out"] for b in range(B)], axis=0)
    if _trace:
        kernel.last_results = res
    return out.astype(np.float32)


# revision 45
# speedup vs baseline: 1.0489x; 1.0489x over previous
"""Trainium2 Bass kernel for a single attention head with query-axis softmax.

Reference semantics (per batch b):
    k = x @ Wk; q = x @ Wq; v = x @ Wv                 # [T, H]
    wei = (q @ k^T) * E**-0.5                          # [T(query), T(key)]
    wei = where(tril, wei, -inf)                       # causal: keep s <= t
    p = softmax(wei, axis=0 over query t)              # NOTE: query axis!
    out = p @ v                                        # [T, H]

Because the softmax normalizes over the query axis t (per key column s),
out[t,h] = sum_s E[t,s] * v[s,h] / d[s] with E[t,s] = exp(wei[t,s])
(zero for s > t) and d[s] = sum_t E[t,s].  The kernel computes E^T tiles
([s on partitions, t free]) so d is a free-axis row sum (fused into the
exp instruction via accum_out), scales v rows by 1/d, and accumulates
out^T on PE.  Cross-partition layout fixes (v^T -> v, out^T -> out) are
PE transposes against a DMA-loaded identity.

PE-array packing: the S matmuls contract over only H=64 partitions, so
two consecutive t-blocks run CONCURRENTLY in the two row-halves of the
128x128 array (row_grp h0/h64).  k^T is duplicated into both partition
halves for free by packing the k projection weights as [Wk|Wk]; q lands
in the partition half matching its block parity by alternating the
packed projection weights [Wq|Wv] (even steps) / [Wv|Wq] (odd steps).
The AV matmuls already pair the same way in the column halves (out^T
rows 0:64 / 64:128 by block parity).

Sharding: batch dim (8) across the 8 NeuronCores, weights replicated.
The host passes x pre-transposed per batch (xT[b] = x[b].T) in bf16 so
no on-device transpose of the big activation tensor is needed; matmul
operands are bf16 with fp32 PSUM accumulation.
"""

import numpy as np
import ml_dtypes

import concourse.bass as bass
import concourse.tile as tile
from concourse import bacc, mybir
from concourse import bass_utils

B, T, E, H = 8, 2048, 1024, 64
P = 128                       # partitions
CB = 512                      # column block (t) width
NE = E // P                   # 8 contraction chunks for projections
NJ = T // CB                  # 4 column blocks
NI = T // P                   # 16 s-chunks
SCALE = float(E) ** -0.5      # note: embed**-0.5, not head_size**-0.5
MASK_NEG = -1.0e30
F32 = mybir.dt.float32
F32R = mybir.dt.float32r
BF16 = mybir.dt.bfloat16
X = mybir.AxisListType.X
EXP = mybir.ActivationFunctionType.Exp


def _qhalf(j):
    """Partition half (0 or 64) where q block j lives (by step parity)."""
    return H if (j % 2) else 0


def _vhalf(j):
    return 0 if (j % 2) else H


def _emit(tc, xT_d, wA_even_d, wA_odd_d, wkk_d, masks_d, identb_d, out_d):
    nc = tc.nc
    from contextlib import ExitStack

    with ExitStack() as ctx:
        singles = ctx.enter_context(tc.tile_pool(name="singles", bufs=1))
        xpool = ctx.enter_context(tc.tile_pool(name="x", bufs=1))
        epool = ctx.enter_context(tc.tile_pool(name="erow", bufs=9))
        dpool = ctx.enter_context(tc.tile_pool(name="dsmall", bufs=12))
        vpool = ctx.enter_context(tc.tile_pool(name="vrow", bufs=9))
        # one unified 3-slot rotation for ALL 2-bank psum tiles (S rows,
        # proj chains, transposes, finale): 3x2 banks + pout 2 = 8 banks.
        # A proj chain holds one slot for its step; the other two give the
        # S pipeline a true row of lookahead in every phase.
        ps = ctx.enter_context(tc.tile_pool(name="ps", bufs=3, space="PSUM"))
        pout = ctx.enter_context(tc.tile_pool(name="pout", bufs=1, space="PSUM"))

        # x^T viewed as [partition, e-chunk, t].  The j=3 column is split
        # into two DMAs (own tiles) so the first projection chain can
        # start as soon as e-chunks 0..3 land; remaining columns follow
        # in consumption order on the same HWDGE ring (FIFO drain).
        # junk memset first: nothing may delay the PE warmup matmuls
        junk = singles.tile([P, CB], BF16)
        nc.gpsimd.memset(junk[:], 1.0)

        # ALL inputs go on ONE HWDGE ring (sync) in exact priority order:
        # HBM bandwidth is the limit, so queue-splitting only scrambles
        # arrival order (queues round-robin at packet granularity).  The
        # ring drains FIFO, so each tensor lands as early as possible
        # given everything more urgent before it.
        xT_v = xT_d.rearrange("(ne p) t -> p ne t", p=P)
        wA_odd = singles.tile([P, NE * 2 * H], BF16)
        nc.sync.dma_start(out=wA_odd[:], in_=wA_odd_d[:])
        xts3a = xpool.tile([P, 4, CB], BF16, tag="xt3a", name="xts3a")
        nc.sync.dma_start(out=xts3a[:], in_=xT_v[:, 0:4, 3 * CB :])
        wkk = singles.tile([P, NE * 2 * H], BF16)
        nc.sync.dma_start(out=wkk[:], in_=wkk_d[:])
        xts3b = xpool.tile([P, NE - 4, CB], BF16, tag="xt3b", name="xts3b")
        nc.sync.dma_start(out=xts3b[:], in_=xT_v[:, 4:, 3 * CB :])
        wA_even = singles.tile([P, NE * 2 * H], BF16)
        nc.sync.dma_start(out=wA_even[:], in_=wA_even_d[:])
        masks_sb = singles.tile([P, 4 * P], F32)
        nc.sync.dma_start(out=masks_sb[:], in_=masks_d[:])
        identb = singles.tile([P, P], BF16)
        nc.sync.dma_start(out=identb[:], in_=identb_d[:])
        xts2 = xpool.tile([P, NE, CB], BF16, tag="xt2", name="xts2")
        nc.sync.dma_start(out=xts2[:], in_=xT_v[:, :, 2 * CB : 3 * CB])
        xtsr = xpool.tile([P, NE, 2 * CB], BF16, tag="xtr", name="xtsr")
        nc.sync.dma_start(out=xtsr[:], in_=xT_v[:, :, 0 : 2 * CB])

        def x_rhs(e, j):
            if j == 3:
                return xts3a[:, e, :] if e < 4 else xts3b[:, e - 4, :]
            if j == 2:
                return xts2[:, e, :]
            return xtsr[:, e, j * CB : (j + 1) * CB]

        # dummy matmuls while the first DMAs land: primes the PE
        # activity monitor so the real chains start at 2.4 GHz, not 1.2
        pwarm = ps.tile([P, 2 * CB], F32, tag="ps", name="pwarm")
        NWARM = 9
        for w in range(NWARM):
            nc.tensor.matmul(
                pwarm[:, 0:CB],
                lhsT=junk[:, 0:P],
                rhs=junk[:],
                start=(w == 0),
                stop=(w == NWARM - 1),
            )
        # tiny dummy exp: walrus hoists the ~1.3us ACT table load here
        # (ACT is idle) instead of paying it before the first real exp
        dwarm = dpool.tile([P, 1], F32, tag="dwarm")
        nc.scalar.activation(out=dwarm[:], in_=junk[:, 0:1], func=EXP)

        # persistent activations: q blocks / k^T copies live in the
        # partition half given by block parity (k^T in BOTH halves)
        q_sb = singles.tile([P, T], BF16)
        kT_sb = singles.tile([P, T], BF16)
        vT_sb = singles.tile([P, T], BF16)   # v^T chunk j in half _vhalf(j)
        outT_sb = singles.tile([P, T // 2], BF16)  # rows 0:64 jj even, 64:128 odd

        # out^T accumulators packed 2 per bank: jj even rows 0:64, odd 64:128.
        # Accumulation groups on disjoint partition ranges of one bank are
        # fine on HW (per-element has_written); skip the sim's coarse check.
        pout_tiles = [
            pout.tile([P, CB], F32, tag=f"pt{a}", name=f"pt{a}") for a in range(2)
        ]

        def pout_slice(jj, c0, c1):
            rb = H * (jj % 2)
            return pout_tiles[jj // 2][rb : rb + H, c0:c1]

        # deferred AV emission (lag one row behind S so PE never waits on
        # the d / v' chain): each entry = (r, d0, erow, vi, j_of_row)
        pending_av = []

        def _av_one(rj, d0, erow, vi, jj):
            c = (jj - rj[1]) * CB
            lo = d0 if jj == rj[1] else 0
            nc.tensor.matmul(
                pout_slice(jj, lo, CB),
                lhsT=vi[:],
                rhs=erow[:, c + lo : c + CB],
                start=(jj == rj[1] and rj[0] == 0),
                stop=(rj[1] == 0 and rj[0] == 3),
                skip_group_check=True,
            )

        def flush_av(final):
            if final:
                # group by output bank so bank A closes early and its
                # finale transposes overlap bank B's last matmuls
                rows = list(pending_av)
                pending_av.clear()
                for jj in range(NJ):
                    for rj, d0, erow, vi in rows:
                        if jj >= rj[1]:
                            _av_one(rj, d0, erow, vi, jj)
                    if jj % 2 == 1:
                        finale_bank(jj // 2)
                return
            rj, d0, erow, vi = pending_av.pop(0)
            for jj in range(rj[1], NJ):
                _av_one(rj, d0, erow, vi, jj)

        # projection matmul emission is spread through the PREVIOUS step's
        # rows so the PE instruction stream stays dense (HAM stays warm).
        # Chain A ([Wq|Wv] or [Wv|Wq] by parity) and chain B ([Wk|Wk])
        # interleave so the e-chunk DMA chase at j=3 has a longer bridge.
        def proj_thunks(j):
            pproj = ps.tile([P, 2 * CB], F32, tag="ps", name="pproj")
            wA = wA_odd if (j % 2) else wA_even
            thunks = []
            for e in range(NE):
                thunks.append(
                    lambda e=e: nc.tensor.matmul(
                        pproj[:, 0:CB],
                        lhsT=wA[:, e * 2 * H : (e + 1) * 2 * H],
                        rhs=x_rhs(e, j),
                        start=(e == 0),
                        stop=(e == NE - 1),
                    )
                )
                thunks.append(
                    lambda e=e: nc.tensor.matmul(
                        pproj[:, CB : 2 * CB],
                        lhsT=wkk[:, e * 2 * H : (e + 1) * 2 * H],
                        rhs=x_rhs(e, j),
                        start=(e == 0),
                        stop=(e == NE - 1),
                    )
                )
            return pproj, thunks

        def proj_casts(j, pproj, use_act=False):
            # vT first (feeds the next step's up-front transposes); the
            # initial (j=3) call splits across ACT+DVE since ACT is idle
            t0 = j * CB
            qh, vh = _qhalf(j), _vhalf(j)
            nc.vector.tensor_copy(
                vT_sb[vh : vh + H, t0 : t0 + CB], pproj[vh : vh + H, 0:CB]
            )
            nc.scalar.copy(kT_sb[:, t0 : t0 + CB], pproj[:, CB : 2 * CB])
            nc.vector.tensor_copy(
                q_sb[qh : qh + H, t0 : t0 + CB], pproj[qh : qh + H, 0:CB]
            )

        # --- main pipeline: column blocks in descending order --------------
        # chains of pending projection matmuls, dripped through the row
        # loop; when one drains its casts are emitted immediately and the
        # next chain (if its x data can be in flight) is queued so late
        # steps carry less projection work
        next_proj = []
        drip_state = {"pproj": None, "j": None, "cast_done": True}

        def start_chain(j):
            pproj_c, thunks = proj_thunks(j)
            next_proj.extend(thunks)
            drip_state.update(pproj=pproj_c, j=j, cast_done=False)

        def drip_proj(k):
            for _ in range(min(k, len(next_proj))):
                next_proj.pop(0)()
            if not next_proj and not drip_state["cast_done"]:
                # chain fully emitted: emit its casts right away so the
                # next step's S matmuls find q/kT ready at the boundary
                drip_state["cast_done"] = True
                proj_casts(drip_state["j"], drip_state["pproj"])
                if drip_state["j"] == 1:
                    # queue the j=0 chain now: its x data (xtsr) is long
                    # landed and j=1's rows are ACT-bound without it
                    start_chain(0)

        pproj, thunks = proj_thunks(3)
        for t in thunks:
            t()
        proj_casts(3, pproj, use_act=True)

        for j in reversed(range(NJ)):
            if j > 0 and drip_state["cast_done"] and drip_state["j"] != j - 1:
                start_chain(j - 1)

            # transpose this step's four v chunks to natural layout up
            # front (PE, row-half-tiled) and stage them in SBUF.  Keeps
            # the per-row chain off the ps-pool rotation: each row then
            # allocates only its S tile, so the pool gives a true row of
            # lookahead instead of serializing S behind the previous
            # row's exp.
            vh = _vhalf(j)
            vi_raws = []
            for r in range(4):
                s0 = (4 * j + r) * P
                pvt = ps.tile([P, 2 * CB], BF16, tag="ps", name="pvt")
                nc.tensor.transpose(
                    pvt[:, 0:H],
                    vT_sb[vh : vh + H, s0 : s0 + P],
                    identb[vh : vh + H, vh : vh + H],
                )
                vr = vpool.tile([P, H], BF16, tag="vr", name="vi_raw")
                # ACT does the psum->sbuf evacuation: the DVE queue is
                # congested at step boundaries (proj casts + masks) while
                # ACT idles between the previous step's last exp and this
                # step's first
                nc.scalar.copy(vr[:], pvt[:, 0:H])
                vi_raws.append(vr)

            # rows i = 4j .. 4j+3 of E^T are now computable in full
            for r in range(4):
                i = 4 * j + r
                s0 = i * P
                d0 = r * P  # first unmasked column of the diagonal block
                nblk = NJ - j
                # flush due AV matmuls BEFORE this row's S pair: their
                # operands are long ready, so they fill the PE bubble
                # while S waits for its psum slot instead of queuing
                # behind it (engines run strictly in order).  At a step
                # boundary (r=0) flush only the OLDEST pending row: it
                # fills the boundary bubble, while the previous row's vi
                # is still in flight and would stall the queue.
                if r == 0:
                    if len(pending_av) >= 2:
                        flush_av(False)
                else:
                    while len(pending_av) >= 2:
                        flush_av(False)
                erow = epool.tile([P, T], BF16)
                dparts = dpool.tile([P, 2], F32, tag="dparts")
                # pair layout: consecutive blocks share a 2-bank tile
                # (opposite parity -> concurrent S).  At j=1 the diagonal
                # block goes ALONE first: its (short, masked) activation
                # releases the psum slot early and shortens the row spine.
                if j == 1:
                    pairs = [(1, CB), (2, 2 * CB)]
                else:
                    pairs = [
                        (j + 2 * p, CB * min(2, NJ - (j + 2 * p)))
                        for p in range((nblk + 1) // 2)
                    ]
                npair = len(pairs)
                use_accum = True
                for pair, (jj0, w) in enumerate(pairs):
                    pst = ps.tile([P, 2 * CB], F32, tag="ps")
                    for u in range(w // CB):
                        jj = jj0 + u
                        lo = d0 if jj == j else 0
                        h = H * (jj % 2)
                        nc.tensor.matmul(
                            pst[:, u * CB + lo : (u + 1) * CB],
                            lhsT=kT_sb[h : h + H, s0 : s0 + P],
                            rhs=q_sb[h : h + H, jj * CB + lo : (jj + 1) * CB],
                            start=True,
                            stop=True,
                        )
                    # step 3 row 12: don't drip yet — the j=2 chain waits
                    # on the xts2 DMA and would head-of-line block the
                    # row matmuls behind it in the PE queue
                    if not (j == 3 and r < 1):
                        drip_proj(3)
                    lo = d0 if pair == 0 else 0
                    if pair == 0:
                        # additive -1e30 triangle on the partial subblock
                        nc.vector.tensor_add(
                            pst[:, lo : lo + P],
                            pst[:, lo : lo + P],
                            masks_sb[:, r * P : (r + 1) * P],
                        )
                    c = (jj0 - j) * CB
                    nc.scalar.activation(
                        out=erow[:, c + lo : c + w],
                        in_=pst[:, lo:w],
                        func=EXP,
                        scale=SCALE,
                        accum_out=(
                            dparts[:, pair : pair + 1] if use_accum else None
                        ),
                    )

                # d = sum over the row; 1/d feeds the v' scale
                dinv = dpool.tile([P, 1], F32, tag="dinv")
                if not use_accum:
                    dsum = dpool.tile([P, 1], F32, tag="dsum")
                    nc.vector.reduce_sum(
                        dsum[:], erow[:, d0 : CB * nblk], axis=X
                    )
                    nc.vector.reciprocal(dinv[:], dsum[:])
                elif npair > 1:
                    dsum = dpool.tile([P, 1], F32, tag="dsum")
                    nc.vector.reduce_sum(dsum[:], dparts[:, 0:npair], axis=X)
                    nc.vector.reciprocal(dinv[:], dsum[:])
                else:
                    nc.vector.reciprocal(dinv[:], dparts[:, 0:1])

                vi = vpool.tile([P, H], BF16, tag="vi", name="vi")
                nc.vector.tensor_scalar_mul(vi[:], vi_raws[r][:], dinv[:])

                if not (j == 3 and r < 1):
                    drip_proj(3)
                pending_av.append(((r, j), d0, erow, vi))

            # drain remaining next-step projection matmuls (+ casts)
            drip_proj(len(next_proj))

        # finale per pout bank: cast out of PSUM, transpose to natural
        # layout, stage for the single store DMA
        onf = singles.tile([P, NI, H], F32)

        out_v = out_d.rearrange("(c p) h -> p c h", p=P)

        def finale_bank(a):
            nc.vector.tensor_copy(
                outT_sb[:, a * CB : (a + 1) * CB], pout_tiles[a][:]
            )
            # two psum tiles, 4 transposes each at distinct column
            # offsets, one wide cast per tile — avoids the serial
            # transpose/cast rotation of the old per-chunk path
            for half in range(2):
                pso = ps.tile([P, 2 * CB], BF16, tag="ps", name="pso")
                for cc in range(4):
                    c = 8 * a + 4 * half + cc
                    jj = c // 4
                    rb = H * (jj % 2)
                    col = (jj // 2) * CB + (c % 4) * P
                    nc.tensor.transpose(
                        pso[:, cc * H : (cc + 1) * H],
                        outT_sb[rb : rb + H, col : col + P],
                        identb[rb : rb + H, rb : rb + H],
                    )
                nc.vector.tensor_copy(
                    onf[:, 8 * a + 4 * half : 8 * a + 4 * half + 4, :],
                    pso[:, 0 : 4 * H],
                )
                # store each quarter as soon as it's staged so the last
                # DMA (and its ~2us completion receipt) is small and early
                nc.sync.dma_start(
                    out=out_v[:, 8 * a + 4 * half : 8 * a + 4 * half + 4, :],
                    in_=onf[:, 8 * a + 4 * half : 8 * a + 4 * half + 4, :],
                )

        flush_av(True)


def _build_program():
    nc = bacc.Bacc("TRN2", target_bir_lowering=False, debug=False, num_devices=B)
    xT_d = nc.dram_tensor("xT", [E, T], BF16, kind="ExternalInput").ap()
    wA_even_d = nc.dram_tensor(
        "wA_even", [P, NE * 2 * H], BF16, kind="ExternalInput"
    ).ap()
    wA_odd_d = nc.dram_tensor(
        "wA_odd", [P, NE * 2 * H], BF16, kind="ExternalInput"
    ).ap()
    wkk_d = nc.dram_tensor("wkk", [P, NE * 2 * H], BF16, kind="ExternalInput").ap()
    masks_d = nc.dram_tensor("masks", [P, 4 * P], F32, kind="ExternalInput").ap()
    identb_d = nc.dram_tensor("identb", [P, P], BF16, kind="ExternalInput").ap()
    out_d = nc.dram_tensor("out", [T, H], F32, kind="ExternalOutput").ap()
    with tile.TileContext(nc) as tc:
        _emit(tc, xT_d, wA_even_d, wA_odd_d, wkk_d, masks_d, identb_d, out_d)
    nc.compile()
    return nc


def _host_masks():
    """[128, 4*128]: additive triangle mask r at cols [128r, 128(r+1))."""
    m = np.full((P, 4 * P), MASK_NEG, dtype=np.float32)
    p = np.arange(P)[:, None]
    f = np.arange(P)[None, :]
    for r in range(4):
        m[:, r * P : (r + 1) * P][f >= p] = 0.0
    return m


def _host_inputs(x, Wk, Wq, Wv):
    bf = ml_dtypes.bfloat16
    x = np.asarray(x, dtype=np.float32)
    xT = np.ascontiguousarray(np.transpose(x, (0, 2, 1))).astype(bf)  # [B, E, T]

    def pack_w(*ws):
        # [E, h_tot] (concat) -> [128, NE * h_tot]: chunk e at cols e*h_tot
        w = np.concatenate([np.asarray(a, np.float32) for a in ws], axis=1)
        h = w.shape[1]
        return np.ascontiguousarray(
            w.reshape(NE, P, h).transpose(1, 0, 2).reshape(P, NE * h)
        ).astype(bf)

    wA_even = pack_w(Wq, Wv)   # q in partitions 0:64 (even blocks)
    wA_odd = pack_w(Wv, Wq)    # q in partitions 64:128 (odd blocks)
    wkk = pack_w(Wk, Wk)       # k^T duplicated into both halves
    masks = _host_masks()
    identb = np.eye(P, dtype=np.float32).astype(bf)
    return [
        {
            "xT": xT[b],
            "wA_even": wA_even,
            "wA_odd": wA_odd,
            "wkk": wkk,
            "masks": masks,
            "identb": identb,
        }
        for b in range(B)
    ]


def _ensure_axon_ntff_hook():
    """The agent image's antenv lacks axon_hooks; synthesize it so
    run_bass_kernel_spmd's trace path can find the NTFF profile hook."""
    import sys
    import types

    if "antenv.axon_hooks" in sys.modules:
        return
    try:
        import antenv

        mod = types.ModuleType("antenv.axon_hooks")
        mod._hook = None

        def set_axon_ntff_profile_hook(h):
            mod._hook = h

        def get_axon_ntff_profile_hook():
            return mod._hook

        mod.set_axon_ntff_profile_hook = set_axon_ntff_profile_hook
        mod.get_axon_ntff_profile_hook = get_axon_ntff_profile_hook
        sys.modules["antenv.axon_hooks"] = mod
        antenv.axon_hooks = mod

        from trn_agent_boot.trn_boot import _ntff_profile_via_ctypes

        hook = _ntff_profile_via_ctypes("/opt/axon/libaxon_pjrt.so")
        if hook is not None:
            mod._hook = hook
    except Exception as e:  # degrade to untraced run
        print(f"NTFF hook setup failed ({e}); tracing will be skipped")


def kernel(x, Wk, Wq, Wv, _trace=False, _trace_kwargs=None):
    if _trace:
        _ensure_axon_ntff_hook()
    in_maps = _host_inputs(x, Wk, Wq, Wv)
    nc = _build_program()
    res = bass_utils.run_bass_kernel_spmd(
        nc, in_maps, list(range(B)), trace=_trace, **(_trace_kwargs or {})
    )
    out = np.stack([res.results[b]["out"] for b in range(B)], axis=0)
    if _trace:
        kernel.last_results = res
    return out.astype(np.float32)
